# revision 15
# baseline (speedup 1.0000x reference)
"""Bezier Gaussian-splat raster kernel for 8 Trainium2 NeuronCores.

Reference computation (RES=1024, STEPS=256, SIGMA=0.01):
    curve = bezier(control_points)            # (2, 256)
    Ex[a,s] = exp(-(g[a]-x[s])^2 / (2 sigma^2))   # (1024, 256)
    Ey[b,s] = exp(-(g[b]-y[s])^2 / (2 sigma^2))
    OUT     = (Ey @ Ex^T) / 256               # (1024, 1024)  == raster.T

Sharding: 4 row-blocks x 2 col-blocks = 8 cores. Core i handles output rows
[256*(i//2), +256) and cols [512*(i%2), +512).

Per-core device pipeline:
  1. curve points via a K=3 matmul against the Bezier basis (s on partitions)
  2. per-point coefficients B = 2c*x, C2 = -c*x^2 (DVE)
  3. Gaussian exponent args = B[s]*g[n] + (-c*g[n]^2 - ln(S)/2) via DVE
     scalar_tensor_tensor against broadcast grid constants (fp32)
  4. exp on ACT with per-partition bias C2[s], output fp16
  5. 256-contraction matmul (2 s-chunks x 2 m-chunks, N=512) -> PSUM fp32
  6. PSUM -> SBUF copies (ACT + DVE in parallel), DMA out

The 1/STEPS scale and the -c*g^2 completion terms are folded into the
exponent constants, so the matmul result is the final output.
"""

import math

import numpy as np

import concourse.bacc as bacc
import concourse.bass as bass
import concourse.mybir as mybir
import concourse.tile as tile
from concourse.bass_utils import run_bass_kernel_spmd

RES = 1024
STEPS = 256
SIGMA = 0.01
INV2S2 = 1.0 / (2.0 * SIGMA * SIGMA)  # 5000.0
HALF_LN_S = math.log(STEPS) / 2.0

R_BLK = 4  # row blocks
C_BLK = 2  # col blocks
MROWS = RES // R_BLK  # 256 output rows per core
NCOLS = RES // C_BLK  # 512 output cols per core
N_CORES = 8

F32 = mybir.dt.float32
F16 = mybir.dt.float16

# matmul-input dtype for the exp outputs (accuracy/perf tradeoff)
G_DTYPE = F16

_CACHE: dict = {}


def _build_nc() -> bass.Bass:
    nc = bacc.Bacc(
        "TRN2",
        target_bir_lowering=False,
        debug=False,
        enable_asserts=False,
        num_devices=N_CORES,
    )

    # bez basis [3, 0:256] and control points [3, 256:258] packed in one
    # tensor so the curve matmul needs only one DMA-completion wait (fp32
    # matmuls self-load weights; the LW struct carries at most one wait).
    bezcp = nc.dram_tensor("bezcp", [3, STEPS + 2], F32, kind="ExternalInput").ap()
    # grid broadcast [:, :NCOLS] and -c*g^2 - ln(S)/2 broadcast [:, NCOLS:]
    # packed per axis so each arg stt waits on a single DMA completion.
    gxc = nc.dram_tensor("gxc", [128, 2 * NCOLS], F32, kind="ExternalInput").ap()
    gyc = nc.dram_tensor("gyc", [128, 2 * MROWS], F32, kind="ExternalInput").ap()
    out = nc.dram_tensor("out", [MROWS, NCOLS], F32, kind="ExternalOutput").ap()

    MULT = mybir.AluOpType.mult
    ADD = mybir.AluOpType.add
    EXP = mybir.ActivationFunctionType.Exp

    with tile.TileContext(nc) as tc:
        with (
            tc.tile_pool(name="const", bufs=1) as cpool,
            tc.tile_pool(name="work", bufs=1) as wpool,
            tc.tile_pool(name="ps", bufs=1, space="PSUM") as ppool,
        ):
            # --- early ACT table load trigger (exp set) -------------------
            scratch = cpool.tile([128, 2], F32)
            nc.vector.memset(scratch[:], 0.0)
            nc.scalar.activation(scratch[:, 1:2], scratch[:, 0:1], EXP)

            # --- PE warmup dummies (HAM un-throttle) ----------------------
            wdum = cpool.tile([128, 128], G_DTYPE)
            nc.vector.memset(wdum[:], 0.25)
            pdum = ppool.tile([128, 128], F32)
            n_warm_pre = 6
            n_warm_post = 14
            for _ in range(n_warm_pre):
                nc.tensor.matmul(pdum[:], wdum[:], wdum[:], start=True, stop=True)

            # --- inputs -----------------------------------------------------
            bezcp_sb = cpool.tile([3, STEPS + 2], F32)
            nc.sync.dma_start(bezcp_sb[:], bezcp)
            gxc_sb = cpool.tile([128, 2 * NCOLS], F32)
            nc.sync.dma_start(gxc_sb[:], gxc)
            gyc_sb = cpool.tile([128, 2 * MROWS], F32)
            nc.sync.dma_start(gyc_sb[:], gyc)
            gx_sb = gxc_sb[:, 0:NCOLS]
            gcx_sb = gxc_sb[:, NCOLS : 2 * NCOLS]
            gy_sb = gyc_sb[:, 0:MROWS]
            gcy_sb = gyc_sb[:, MROWS : 2 * MROWS]

            # tiny DVE reads that fold the gxc/gyc DMA completions into the
            # DVE vector clock, so the arg stt ops below need only a single
            # (same-engine) sync wait (STT struct carries at most one).
            touch = wpool.tile([128, 2], F32)
            nc.vector.tensor_copy(touch[:, 0:1], gxc_sb[:, 0:1])
            nc.vector.tensor_copy(touch[:, 1:2], gyc_sb[:, 0:1])

            # --- curve points: pcurve[p, 2k+d] = coord d of point s=128k+p --
            pcurve = ppool.tile([128, 4], F32)
            nc.tensor.matmul(
                pcurve[:, 0:2],
                bezcp_sb[:, 0:128],
                bezcp_sb[:, 256:258],
                start=True,
                stop=True,
            )
            nc.tensor.matmul(
                pcurve[:, 2:4],
                bezcp_sb[:, 128:256],
                bezcp_sb[:, 256:258],
                start=True,
                stop=True,
            )

            for _ in range(n_warm_post):
                nc.tensor.matmul(pdum[:], wdum[:], wdum[:], start=True, stop=True)

            # --- per-point coefficients -------------------------------------
            # bc[:, 0:4] = B = 2c * xy ; bc[:, 4:8] = C2 = -c * xy^2
            xy = wpool.tile([128, 4], F32)
            nc.vector.tensor_copy(xy[:], pcurve[:])
            bc = wpool.tile([128, 8], F32)
            nc.vector.tensor_scalar(bc[:, 0:4], xy[:], 2.0 * INV2S2, None, MULT)
            nc.vector.scalar_tensor_tensor(
                bc[:, 4:8], xy[:], -INV2S2, xy[:], MULT, MULT
            )

            # --- exponent args + exp ---------------------------------------
            # column layout in pcurve/bc: 2k+0 = x chunk k, 2k+1 = y chunk k
            gxe = []
            gye = []
            for k in range(2):
                argx = wpool.tile([128, NCOLS], F32, tag=f"argx{k}")
                nc.vector.scalar_tensor_tensor(
                    argx[:], gx_sb[:], bc[:, 2 * k : 2 * k + 1], gcx_sb[:], MULT, ADD
                )
                ex = wpool.tile([128, NCOLS], G_DTYPE, tag=f"gxe{k}")
                nc.scalar.activation(
                    ex[:], argx[:], EXP, bias=bc[:, 4 + 2 * k : 5 + 2 * k]
                )
                gxe.append(ex)

                argy = wpool.tile([128, MROWS], F32, tag=f"argy{k}")
                nc.vector.scalar_tensor_tensor(
                    argy[:], gy_sb[:], bc[:, 2 * k + 1 : 2 * k + 2], gcy_sb[:], MULT, ADD
                )
                ey = wpool.tile([128, MROWS], G_DTYPE, tag=f"gye{k}")
                nc.scalar.activation(
                    ey[:], argy[:], EXP, bias=bc[:, 5 + 2 * k : 6 + 2 * k]
                )
                gye.append(ey)

            # --- big matmul: OUT[m, n] = sum_s Ey[s, m] * Ex[s, n] ----------
            pouts = [
                ppool.tile([128, NCOLS], F32, tag=f"pout{m}", name=f"pout{m}")
                for m in range(2)
            ]
            for k in range(2):
                for m in range(2):
                    nc.tensor.matmul(
                        pouts[m][:],
                        gye[k][:, 128 * m : 128 * (m + 1)],
                        gxe[k][:],
                        start=(k == 0),
                        stop=(k == 1),
                        skip_group_check=True,
                    )

            # --- evacuate + store ------------------------------------------
            out0 = wpool.tile([128, NCOLS], F32, tag="out0")
            nc.scalar.copy(out0[:], pouts[0][:])
            nc.sync.dma_start(out[0:128, :], out0[:])
            out1 = wpool.tile([128, NCOLS], F32, tag="out1")
            nc.vector.tensor_copy(out1[:], pouts[1][:])
            nc.sync.dma_start(out[128:256, :], out1[:])

    nc.compile()
    return nc


def _static_inputs():
    """Per-core constant input arrays (independent of control_points)."""
    grid = (np.arange(RES, dtype=np.float64)) / RES

    u = np.linspace(0.0, 1.0, STEPS, dtype=np.float32).astype(np.float64)
    v = (np.arange(STEPS, dtype=np.float64)) / STEPS
    c0 = (1.0 - u) * (1.0 - v)
    c1 = u + v - 2.0 * u * v
    c2 = u * v
    bez = np.stack([c0, c1, c2]).astype(np.float32)  # [3, STEPS]

    gc = -INV2S2 * grid * grid - HALF_LN_S  # [RES]

    per_core = []
    for i in range(N_CORES):
        r, c = i // C_BLK, i % C_BLK
        r0, c0_ = r * MROWS, c * NCOLS
        gx_row = grid[c0_ : c0_ + NCOLS].astype(np.float32)
        gcx_row = gc[c0_ : c0_ + NCOLS].astype(np.float32)
        gy_row = grid[r0 : r0 + MROWS].astype(np.float32)
        gcy_row = gc[r0 : r0 + MROWS].astype(np.float32)
        gxc_row = np.concatenate([gx_row, gcx_row])
        gyc_row = np.concatenate([gy_row, gcy_row])
        per_core.append(
            {
                "gxc": np.ascontiguousarray(
                    np.broadcast_to(gxc_row, (128, 2 * NCOLS))
                ),
                "gyc": np.ascontiguousarray(
                    np.broadcast_to(gyc_row, (128, 2 * MROWS))
                ),
            }
        )
    return bez, per_core


def _get_cached():
    if "nc" not in _CACHE:
        _CACHE["nc"] = _build_nc()
        _CACHE["bez"], _CACHE["static"] = _static_inputs()
    return _CACHE["nc"], _CACHE["bez"], _CACHE["static"]


def kernel(control_points: np.ndarray, _trace: bool = False):
    nc, bez, static = _get_cached()
    cp = np.ascontiguousarray(control_points, dtype=np.float32)
    assert cp.shape == (3, 2)
    bezcp = np.ascontiguousarray(
        np.concatenate([bez, cp], axis=1), dtype=np.float32
    )

    in_maps = [{"bezcp": bezcp, **static[i]} for i in range(N_CORES)]
    res = run_bass_kernel_spmd(
        nc, in_maps, core_ids=list(range(N_CORES)), trace=_trace
    )
    _CACHE["last_results"] = res

    full = np.empty((RES, RES), dtype=np.float32)
    for i in range(N_CORES):
        r, c = i // C_BLK, i % C_BLK
        full[r * MROWS : (r + 1) * MROWS, c * NCOLS : (c + 1) * NCOLS] = res.results[
            i
        ]["out"]
    return full
